# revision 1
# baseline (speedup 1.0000x reference)
"""Trainium2 Bass kernel for a 4-bit-quantized linear layer.

Computes y = x @ W^T + bias where W [O, I] is dequantized on-chip from
packed int4 nibbles with a per-group (16 along I) abs-max scale:
    W[o, i] = (q/15) * 2*norm - norm = (2*norm/15) * (q - 7.5)

Distribution: tensor-parallel over output features. Each of the 8 cores
owns O/8 = 2048 output rows (its slice of quantized_weights /
weight_normalization / bias), the input is replicated, and the host
concatenates the per-core [T, 2048] outputs along the feature axis.

Per-core device program:
  1. Dequantize the weight shard to fp16 in [o, i] layout on DVE
     (bitwise nibble extraction + fused (q - 7.5) * a with a = 2*norm/15),
     then one xbar DMA-transpose per 128-row o-tile into a resident
     [i, o] fp16 tensor in SBUF (3D out AP folds the k-tile dim into the
     partition dim, so the per-op HWDGE fixed cost is paid 16x, not 512x).
  2. Stream x in 128-token blocks: fp32->fp16 cast during the (SWDGE) DMA
     load, one xbar DMA-transpose to all 32 x^T k-tiles, and accumulate
     psum[t, o] += xT_k^T @ WT_k over the 32 k-tiles on the TensorEngine
     (x^T k-tile stationary, 4 psum banks of 512 output features each).
  3. Add bias (DVE scalar_tensor_tensor fused with the PSUM->SBUF copy)
     and DMA out.
"""

import numpy as np

import concourse.bass as bass
import concourse.mybir as mybir
from concourse import bacc
from concourse.tile import TileContext

# Full problem shapes (hardcoded; kernel.py must be self-contained).
B, S = 4, 2048
IN_F = 4096
OUT_F = 16384
GROUP = 16
N_CORES = 8
T_FULL = B * S                    # 8192 tokens
O_SH = OUT_F // N_CORES           # 2048 output features per core
G_SH = O_SH * IN_F // GROUP       # 524288 quant groups per core

F16 = mybir.dt.float16
F32 = mybir.dt.float32
I32 = mybir.dt.int32

P = 128  # partitions


def emit_linear4bit(tc, x_ap, qw_ap, wn_ap, bias_ap, y_ap, T, I, O):
    """Emit the per-core program into TileContext tc.

    x:  [T, I]  f32   (replicated input)
    qw: [O*I/16, 8] i32  (this core's group rows; each i32 holds one byte
                          = two nibbles)
    wn: [O*I/16, 1] f32  (per-group scale)
    bias: [O] f32
    y:  [T, O] f32
    """
    nc = tc.nc
    op = mybir.AluOpType

    KT = I // P                   # k tiles (contraction)
    OT = O // P                   # o tiles for dequant
    OC = min(512, O)              # psum chunk along o
    NOC = O // OC
    TB = T // P                   # token blocks
    GPI = I // GROUP              # groups per output row
    BPG = GROUP // 2              # bytes per group (8)

    # ---- resident tensors -------------------------------------------------
    # W^T, fp16, [i (partition within k-tile), k-tile, o] -> 2*KT*O bytes/part
    singles = tc.alloc_tile_pool(name="singles", bufs=1)
    wT = singles.tile([P, KT, O], F16, tag="wT", name="wT")
    bias_rep = singles.tile([P, O], F32, tag="bias_rep", name="bias_rep")

    # bias replicated across partitions via a broadcast-source DMA
    nc.sync.dma_start(bias_rep[:, :], bias_ap[None, :].broadcast_to([P, O]))

    # views of the weight inputs grouped by 128-row o tiles
    # qw rows: g = o * GPI + gi ; o = ot*128 + p
    qw_r = qw_ap.rearrange("(ot p g) b -> ot p (g b)", ot=OT, p=P, g=GPI)
    wn_r = wn_ap.rearrange("(ot p g) one -> ot p (g one)", ot=OT, p=P, g=GPI)

    # ---- phase 1: dequantize + transpose W --------------------------------
    with (
        tc.tile_pool(name="qpool", bufs=2) as qpool,
        tc.tile_pool(name="spool", bufs=2) as spool,
        tc.tile_pool(name="nibpool", bufs=2) as nibpool,
        tc.tile_pool(name="wdpool", bufs=2) as wdpool,
    ):
        for ot in range(OT):
            qt = qpool.tile([P, GPI * BPG], I32, tag="qt")
            nc.sync.dma_start(qt[:, :], qw_r[ot])
            wnt = spool.tile([P, GPI], F32, tag="wnt")
            nc.sync.dma_start(wnt[:, :], wn_r[ot])
            at = spool.tile([P, GPI], F32, tag="at")
            nc.vector.tensor_scalar_mul(at[:, :], wnt[:, :], 2.0 / 15.0)

            lo = nibpool.tile([P, GPI * BPG], I32, tag="nib")
            hi = nibpool.tile([P, GPI * BPG], I32, tag="nib")
            nc.vector.tensor_scalar(lo[:, :], qt[:, :], 15, None, op0=op.bitwise_and)
            nc.vector.tensor_scalar(
                hi[:, :], qt[:, :], 4, None, op0=op.logical_shift_right
            )

            wd = wdpool.tile([P, I], F16, tag="wd")
            wd4 = wd[:, :].rearrange("p (g b t) -> p g b t", g=GPI, b=BPG, t=2)
            a_b = at[:, :, None].broadcast_to([P, GPI, BPG])
            lo_r = lo[:, :].rearrange("p (g b) -> p g b", b=BPG)
            hi_r = hi[:, :].rearrange("p (g b) -> p g b", b=BPG)
            # W = (q - 7.5) * (2*norm/15)
            nc.vector.scalar_tensor_tensor(
                wd4[:, :, :, 0], lo_r, -7.5, a_b, op0=op.add, op1=op.mult
            )
            nc.vector.scalar_tensor_tensor(
                wd4[:, :, :, 1], hi_r, -7.5, a_b, op0=op.add, op1=op.mult
            )

            # one xbar-transpose for the whole o-tile: logical [I, 128] <-
            # [128, I]; out extra dim k folds into the partition dim
            nc.sync.dma_start_transpose(
                wT[:, :, ot * P : (ot + 1) * P], wd[:, :]
            )

    # ---- phase 2: main matmul loop over token blocks ----------------------
    with (
        tc.tile_pool(name="xfpool", bufs=3) as xfpool,
        tc.tile_pool(name="xTpool", bufs=3) as xTpool,
        tc.tile_pool(name="opool", bufs=8) as opool,
        tc.tile_pool(name="pspool", bufs=8, space="PSUM") as pspool,
    ):
        for tb in range(TB):
            trow = slice(tb * P, (tb + 1) * P)
            # fp32 -> fp16 cast during the DMA (SWDGE)
            xf = xfpool.tile([P, I], F16, tag="xf")
            nc.gpsimd.dma_start(xf[:, :], x_ap[trow, :])
            xT = xTpool.tile([P, KT, P], F16, tag="xT")
            nc.sync.dma_start_transpose(xT[:, :, :], xf[:, :])

            ps = []
            for oc in range(NOC):
                pst = pspool.tile([P, OC], F32, tag="ps")
                ps.append(pst)
            for k in range(KT):
                lhs = xT[:, k, :]
                for oc in range(NOC):
                    nc.tensor.matmul(
                        ps[oc][:, :],
                        lhs,
                        wT[:, k, oc * OC : (oc + 1) * OC],
                        start=(k == 0),
                        stop=(k == KT - 1),
                    )
            for oc in range(NOC):
                osb = opool.tile([P, OC], F32, tag="osb")
                nc.vector.scalar_tensor_tensor(
                    osb[:, :],
                    ps[oc][:, :],
                    0.0,
                    bias_rep[:, oc * OC : (oc + 1) * OC],
                    op0=op.add,
                    op1=op.add,
                )
                nc.sync.dma_start(y_ap[trow, oc * OC : (oc + 1) * OC], osb[:, :])

    singles.release()


def build_nc(T=T_FULL, I=IN_F, O=O_SH):
    nc = bacc.Bacc("TRN2", target_bir_lowering=False, debug=False)
    x = nc.dram_tensor("x", [T, I], F32, kind="ExternalInput")
    qw = nc.dram_tensor("qw", [O * I // GROUP, GROUP // 2], I32, kind="ExternalInput")
    wn = nc.dram_tensor("wn", [O * I // GROUP, 1], F32, kind="ExternalInput")
    b = nc.dram_tensor("bias", [O], F32, kind="ExternalInput")
    y = nc.dram_tensor("y", [T, O], F32, kind="ExternalOutput")
    with TileContext(nc) as tc:
        emit_linear4bit(tc, x.ap(), qw.ap(), wn.ap(), b.ap(), y.ap(), T, I, O)
    nc.compile()
    return nc


TRACE = False
LAST_RESULT = None


def kernel(input_tensor, quantized_weights, weight_normalization, bias):
    global LAST_RESULT
    from concourse.bass_utils import run_bass_kernel_spmd

    x = np.ascontiguousarray(
        np.asarray(input_tensor, dtype=np.float32).reshape(T_FULL, IN_F)
    )
    qw = np.asarray(quantized_weights, dtype=np.int32)
    wn = np.asarray(weight_normalization, dtype=np.float32)
    b = np.asarray(bias, dtype=np.float32)

    nc = build_nc()
    in_maps = []
    for c in range(N_CORES):
        in_maps.append(
            {
                "x": x,
                "qw": np.ascontiguousarray(qw[c * G_SH : (c + 1) * G_SH]),
                "wn": np.ascontiguousarray(wn[c * G_SH : (c + 1) * G_SH]),
                "bias": np.ascontiguousarray(b[c * O_SH : (c + 1) * O_SH]),
            }
        )
    res = run_bass_kernel_spmd(nc, in_maps, list(range(N_CORES)), trace=TRACE)
    LAST_RESULT = res
    y = np.concatenate([r["y"] for r in res.results], axis=1)
    return np.ascontiguousarray(y.reshape(B, S, OUT_F), dtype=np.float32)



# revision 29
# speedup vs baseline: 1.2490x; 1.2490x over previous
"""Trainium2 Bass kernel for a 4-bit-quantized linear layer.

Computes y = x @ W^T + bias where W [O, I] is dequantized on-chip from
packed int4 nibbles with a per-group (16 along I) abs-max scale:
    W[o, i] = (q/15) * 2*norm - norm = (2*norm/15) * (q - 7.5)

Distribution: tensor-parallel over output features. Each of the 8 cores
owns O/8 = 2048 output rows (its slice of quantized_weights /
weight_normalization / bias), the input is replicated, and the host
concatenates the per-core [T, 2048] outputs along the feature axis.

Per-core device program (fp8 DoubleRow, 3-pass residual-split):
  The TensorEngine runs fp8e4 matmuls in DoubleRow perf mode: each
  instruction contracts a PAIR of 128-deep k-tiles at 0.5 cycles per
  output row - 4x the fp16 rate. A single fp8 pass quantizes both x and
  W to ~2.6% rms which fails the 2e-2 gate, so both sides are split into
  (hi, lo) e4m3 terms and three full DoubleRow passes are accumulated
  in PSUM:
      y ~= x8@W8hi + x8@W8lo + dx8@W8hi      (dx8@W8lo ~ 0.07%, dropped)
  giving ~2e-3 end-to-end error at 3/4 of the fp16 baseline matmul cost.

  Weight pipeline (software-pipelined in 1024-element k-quarters, DMA
  loads running two o-tiles ahead and the fp8 split lagging two steps so
  no engine stream ever stalls on an upstream chain): DMA the packed
  bytes (int32 storage narrowed to u8 in the DMA), extract both nibble
  planes as u16 word-ops at 2x DVE rate ((w>>4)&0x0f0f / w&0x0f0f),
  dequantize with fused (q-7.5)*a (even nibbles DVE, odd Pool),
  xbar-DMA-transpose the fp16 quarter, then split: W8hi = e4m3(W) on
  Act, W8lo = e4m3(W - W8hi) on DVE (every 4th quarter on Pool). The
  split tensors are resident [i, kt, o], one tile per 512-wide o-chunk
  so chunk matmuls only wait on the four o-tiles they actually read.

  Input pipeline: per 128-token block in k-halves, fp32->fp16 cast in
  the SWDGE DMA load, xbar-transpose, x8 = e4m3(xT) on Act, dx8 =
  e4m3(xT - x8) on DVE. The first NFRONT blocks are staggered into the
  weight pipeline and their chunks emitted oc-column-major, so the
  in-order PE stream starts multiplying as soon as the first o-chunk
  column is dequantized.
"""

import numpy as np

import concourse.bass as bass
import concourse.mybir as mybir
from concourse import bacc
from concourse.tile import TileContext

# Full problem shapes (hardcoded; kernel.py must be self-contained).
B, S = 4, 2048
IN_F = 4096
OUT_F = 16384
GROUP = 16
N_CORES = 8
T_FULL = B * S                    # 8192 tokens
O_SH = OUT_F // N_CORES           # 2048 output features per core
G_SH = O_SH * IN_F // GROUP       # 524288 quant groups per core

F8 = mybir.dt.float8e4
F16 = mybir.dt.float16
F32 = mybir.dt.float32
I32 = mybir.dt.int32
U8 = mybir.dt.uint8
U16 = mybir.dt.uint16

P = 128      # partitions
NFRONT = 3   # token blocks prepared ahead of the weight pipeline
LEAD_OT = 2  # o-tiles the weight DMAs run ahead of compute
SPLIT_LAG = 2  # quarters the fp8 split lags the transpose


def emit_linear4bit(tc, x_ap, qw_ap, wn_ap, bias_ap, y_ap, T, I, O):
    """Emit the per-core program into TileContext tc.

    x:  [T, I]  f32   (replicated input)
    qw: [O*I/16, 8] i32  (this core's group rows; each i32 holds one byte
                          = two nibbles)
    wn: [O*I/16, 1] f32  (per-group scale)
    bias: [O] f32
    y:  [T, O] f32
    """
    nc = tc.nc
    op = mybir.AluOpType
    act = mybir.ActivationFunctionType
    DR = mybir.MatmulPerfMode.DoubleRow

    KT = I // P                   # k tiles (contraction): 32
    KP = KT // 2                  # DoubleRow k-tile pairs: 16
    KH = KT // 2                  # k tiles per half: 16
    KQ = KT // 4                  # k tiles per quarter: 8
    IH = I // 2                   # 2048
    IQ = I // 4                   # 1024
    OT = O // P                   # o tiles: 16
    OC = min(512, O)              # psum chunk along o
    NOC = O // OC                 # 4
    OPC = OC // P                 # o tiles per chunk: 4
    TB = T // P                   # token blocks: 64
    GPI = I // GROUP              # groups per output row: 256
    GQ = GPI // 4                 # groups per quarter: 64
    BPG = GROUP // 2              # bytes per group: 8
    QB = GQ * BPG                 # quant bytes per quarter: 512
    NQ = OT * 4                   # weight quarter-steps: 64

    # ---- resident tensors -------------------------------------------------
    # W8hi/W8lo split per o-chunk so tile-level dependency tracking lets a
    # chunk's matmuls start once its own four o-tiles are dequantized.
    singles = tc.alloc_tile_pool(name="singles", bufs=1)
    w8hi = [
        singles.tile([P, KT, OC], F8, tag=f"w8hi{oc}", name=f"w8hi{oc}")
        for oc in range(NOC)
    ]
    w8lo = [
        singles.tile([P, KT, OC], F8, tag=f"w8lo{oc}", name=f"w8lo{oc}")
        for oc in range(NOC)
    ]
    bias_rep = singles.tile([P, O], F32, tag="bias_rep", name="bias_rep")
    nc.sync.dma_start(bias_rep[:, :], bias_ap[None, :].broadcast_to([P, O]))

    # views of the weight inputs grouped by 128-row o tiles
    # qw rows: g = o * GPI + gi ; o = ot*128 + p
    qw_r = qw_ap.rearrange("(ot p g) b -> ot p (g b)", ot=OT, p=P, g=GPI)
    wn_r = wn_ap.rearrange("(ot p g) one -> ot p (g one)", ot=OT, p=P, g=GPI)

    with (
        tc.tile_pool(name="qpool", bufs=LEAD_OT + 1) as qpool,
        tc.tile_pool(name="wnpool", bufs=LEAD_OT + 1) as wnpool,
        tc.tile_pool(name="atpool", bufs=2) as atpool,
        tc.tile_pool(name="nibpool", bufs=3) as nibpool,
        tc.tile_pool(name="wdpool", bufs=3) as wdpool,
        tc.tile_pool(name="wtpool", bufs=SPLIT_LAG + 2) as wtpool,
        tc.tile_pool(name="xfpool", bufs=4) as xfpool,
        tc.tile_pool(name="xTpool", bufs=2) as xTpool,
        tc.tile_pool(name="x8pool", bufs=NFRONT) as x8pool,
        tc.tile_pool(name="dx8pool", bufs=NFRONT) as dx8pool,
        tc.tile_pool(name="opool", bufs=2) as opool,
        tc.tile_pool(name="pspool", bufs=8, space="PSUM") as pspool,
    ):
        qts = {}   # ot -> qt tile (whole o-tile of packed bytes, u8)
        wnts = {}  # ot -> wn tile
        ats = {}   # ot -> 2*norm/15 tile
        wdTs = {}  # quarter-step -> transposed fp16 tile

        def w_load(ot):
            wnt = wnpool.tile([P, GPI], F32, tag="wnt")
            nc.scalar.dma_start(wnt[:, :], wn_r[ot])
            wnts[ot] = wnt
            qt = qpool.tile([P, GPI * BPG], U8, tag="qt")
            nc.gpsimd.dma_start(qt[:, :], qw_r[ot])
            qts[ot] = qt

        def w_comp(s):
            ot, h = divmod(s, 4)
            if h == 0:
                at = atpool.tile([P, GPI], F32, tag="at")
                nc.vector.tensor_scalar_mul(at[:, :], wnts.pop(ot)[:, :], 2.0 / 15.0)
                ats[ot] = at
            at = ats[ot]
            # both nibble planes via u16 word ops at 2x DVE rate
            qt16 = qts[ot][:, h * QB : (h + 1) * QB].bitcast(U16)
            lo16 = nibpool.tile([P, QB // 2], U16, tag="nib")
            hi16 = nibpool.tile([P, QB // 2], U16, tag="nib")
            nc.vector.tensor_scalar(
                lo16[:, :], qt16, 0x0F0F, None, op0=op.bitwise_and
            )
            nc.vector.tensor_scalar(
                hi16[:, :], qt16, 4, 0x0F0F,
                op0=op.logical_shift_right, op1=op.bitwise_and,
            )
            wd = wdpool.tile([P, IQ], F16, tag="wd")
            wd4 = wd[:, :].rearrange("p (g b t) -> p g b t", g=GQ, b=BPG, t=2)
            a_b = at[:, h * GQ : (h + 1) * GQ, None].broadcast_to([P, GQ, BPG])
            lo_r = lo16[:, :].bitcast(U8).rearrange("p (g b) -> p g b", b=BPG)
            hi_r = hi16[:, :].bitcast(U8).rearrange("p (g b) -> p g b", b=BPG)
            # W = (q - 7.5) * (2*norm/15); both nibble planes on DVE (walrus
            # rejects scalar_tensor_tensor on the Pool engine)
            nc.vector.scalar_tensor_tensor(
                wd4[:, :, :, 0], lo_r, -7.5, a_b, op0=op.add, op1=op.mult
            )
            nc.vector.scalar_tensor_tensor(
                wd4[:, :, :, 1], hi_r, -7.5, a_b, op0=op.add, op1=op.mult
            )
            wdT = wtpool.tile([P, KQ, P], F16, tag="wdT")
            nc.sync.dma_start_transpose(wdT[:, :, :], wd[:, :])
            wdTs[s] = wdT
            if h == 3:
                ats.pop(ot)

        def w_split(s):
            ot, h = divmod(s, 4)
            whi = w8hi[ot // OPC]
            wlo = w8lo[ot // OPC]
            ocol = slice((ot % OPC) * P, (ot % OPC + 1) * P)
            khs = slice(h * KQ, (h + 1) * KQ)
            wdT = wdTs.pop(s)
            nc.scalar.activation(whi[:, khs, ocol], wdT[:, :, :], act.Copy)
            # Pool supports tensor_tensor; put 2/3 of the subtracts there so
            # DVE (nibbles + dequant) and Pool finish together
            sub_eng = nc.vector if s % 3 == 0 else nc.gpsimd
            sub_eng.tensor_sub(wlo[:, khs, ocol], wdT[:, :, :], whi[:, khs, ocol])

        def x_prep(tb):
            """Load + transpose + fp8-split one 128-token block (2 k-halves)."""
            trow = slice(tb * P, (tb + 1) * P)
            x8 = x8pool.tile([P, KT, P], F8, tag="x8")
            dx8 = dx8pool.tile([P, KT, P], F8, tag="dx8")
            for hh in range(0, 4, 2):
                xfs = []
                for h in (hh, hh + 1):
                    xf = xfpool.tile([P, IQ], F16, tag="xf")
                    nc.gpsimd.dma_start(xf[:, :], x_ap[trow, h * IQ : (h + 1) * IQ])
                    xfs.append(xf)
                for j, h in enumerate((hh, hh + 1)):
                    khs = slice(h * KQ, (h + 1) * KQ)
                    xT = xTpool.tile([P, KQ, P], F16, tag="xT")
                    nc.sync.dma_start_transpose(xT[:, :, :], xfs[j][:, :])
                    nc.scalar.activation(x8[:, khs, :], xT[:, :, :], act.Copy)
                    nc.vector.tensor_sub(dx8[:, khs, :], xT[:, :, :], x8[:, khs, :])
            return x8, dx8

        def emit_chunk(tb, oc, x8, dx8):
            """3-pass DoubleRow matmuls + bias + store for one PSUM chunk."""
            trow = slice(tb * P, (tb + 1) * P)
            pst = pspool.tile([P, OC], F32, tag="ps")
            ocs = slice(oc * OC, (oc + 1) * OC)
            whi, wlo = w8hi[oc], w8lo[oc]
            for kp in range(KP):
                ks = slice(2 * kp, 2 * kp + 2)
                for v, (xop, wop) in enumerate(((x8, whi), (x8, wlo), (dx8, whi))):
                    nc.tensor.matmul(
                        pst[:, :],
                        xop[:, ks, :],
                        wop[:, ks, :],
                        start=(kp == 0 and v == 0),
                        stop=(kp == KP - 1 and v == 2),
                        perf_mode=DR,
                    )
            osb = opool.tile([P, OC], F32, tag="osb")
            nc.vector.scalar_tensor_tensor(
                osb[:, :], pst[:, :], 0.0, bias_rep[:, ocs], op0=op.add, op1=op.add
            )
            nc.scalar.dma_start(y_ap[trow, ocs], osb[:, :])

        # ---- emission schedule -------------------------------------------
        xbufs = {}
        for ot in range(LEAD_OT):
            w_load(ot)
        xbufs[0] = x_prep(0)
        xbufs[1] = x_prep(1)
        next_front = 2
        for s in range(NQ + SPLIT_LAG):
            if s < NQ:
                ot, h = divmod(s, 4)
                if h == 0 and ot + LEAD_OT < OT:
                    w_load(ot + LEAD_OT)
                w_comp(s)
            if s >= SPLIT_LAG:
                w_split(s - SPLIT_LAG)
            if s % 12 == 8 and next_front < NFRONT:
                xbufs[next_front] = x_prep(next_front)
                next_front += 1
        while next_front < NFRONT:
            xbufs[next_front] = x_prep(next_front)
            next_front += 1
        # front chunks oc-column-major: column oc waits only on o-tiles
        # 4oc..4oc+3, so the PE stream starts as soon as column 0 is split
        for oc in range(NOC):
            for tb in range(NFRONT):
                emit_chunk(tb, oc, *xbufs[tb])
        xbufs.clear()
        from collections import deque

        pending = deque()
        pending.append(x_prep(NFRONT))
        if NFRONT + 1 < TB:
            pending.append(x_prep(NFRONT + 1))
        for tb in range(NFRONT, TB):
            if tb + 2 < TB:
                pending.append(x_prep(tb + 2))
            x8, dx8 = pending.popleft()
            for oc in range(NOC):
                emit_chunk(tb, oc, x8, dx8)

    singles.release()


def build_nc(T=T_FULL, I=IN_F, O=O_SH):
    nc = bacc.Bacc("TRN2", target_bir_lowering=False, debug=False)
    x = nc.dram_tensor("x", [T, I], F32, kind="ExternalInput")
    qw = nc.dram_tensor("qw", [O * I // GROUP, GROUP // 2], I32, kind="ExternalInput")
    wn = nc.dram_tensor("wn", [O * I // GROUP, 1], F32, kind="ExternalInput")
    b = nc.dram_tensor("bias", [O], F32, kind="ExternalInput")
    y = nc.dram_tensor("y", [T, O], F32, kind="ExternalOutput")
    with TileContext(nc) as tc:
        emit_linear4bit(tc, x.ap(), qw.ap(), wn.ap(), b.ap(), y.ap(), T, I, O)
    nc.compile()
    return nc


TRACE = False
LAST_RESULT = None


def kernel(input_tensor, quantized_weights, weight_normalization, bias):
    global LAST_RESULT
    from concourse.bass_utils import run_bass_kernel_spmd

    x = np.ascontiguousarray(
        np.asarray(input_tensor, dtype=np.float32).reshape(T_FULL, IN_F)
    )
    qw = np.asarray(quantized_weights, dtype=np.int32)
    wn = np.asarray(weight_normalization, dtype=np.float32)
    b = np.asarray(bias, dtype=np.float32)

    nc = build_nc()
    in_maps = []
    for c in range(N_CORES):
        in_maps.append(
            {
                "x": x,
                "qw": np.ascontiguousarray(qw[c * G_SH : (c + 1) * G_SH]),
                "wn": np.ascontiguousarray(wn[c * G_SH : (c + 1) * G_SH]),
                "bias": np.ascontiguousarray(b[c * O_SH : (c + 1) * O_SH]),
            }
        )
    res = run_bass_kernel_spmd(nc, in_maps, list(range(N_CORES)), trace=TRACE)
    LAST_RESULT = res
    y = np.concatenate([r["y"] for r in res.results], axis=1)
    return np.ascontiguousarray(y.reshape(B, S, OUT_F), dtype=np.float32)
